# revision 38
# baseline (speedup 1.0000x reference)
"""Directed message-passing GNN (chemprop-style D-MPNN) on 8 Trainium2 cores.

Strategy (node-range sharding, zero collectives), v2 — bf16 compute:
  - Host sorts edges by target node and splits nodes into 8 contiguous
    ranges of 12500 (edges follow their target's range, ~E/8 per core).
  - All on-chip compute in bf16 (PSUM accumulation stays fp32): one-pass
    matmuls with fast weight load, 2x DVE elementwise, halved DMA bytes.
  - Per core, per 1024-edge chunk: 8 indirect-DMA gathers of x[:, :128]
    rows (256B bf16 each), PE-transposed to feature-major; the remaining 5
    x features ride a host-prepared [19, E] edge stream together with
    edge_attr.  All DEPTH=3 iterations of the message MLP + GRU run
    on-chip; Wm2 is folded into W_ih on the host.  The (gn + r*ghn) gate
    add runs on the PE via an identity-matmul accumulate into the gn PSUM
    bank instead of a 1x-rate PSUM tensor_tensor.
  - Final messages are PE-transposed to edge-major and written to a DRAM
    scratch buffer (bf16) in target-sorted order, 8-edge-packed rows.
  - Segment-sum: per 128-node tile, ONE indirect gather of 128 8-packed
    rows (2KB each) covering the tile's edge span, one-hot matrices built
    with 4x-rate tensor_scalar is_equal(IOTA, seg), then msg.T @ onehot
    accumulated in PSUM -> feature-major node messages.  Node MLP + final
    PE transpose complete the output tile (fp32).
"""

import sys

sys.path.insert(0, "/opt/trn_rl_repo")

import numpy as np
from contextlib import ExitStack

import concourse.bass as bass
import concourse.mybir as mybir
import concourse.tile as tile
from concourse.bass import IndirectOffsetOnAxis
from concourse.bass_utils import run_bass_kernel_spmd

# ---------------------------------------------------------------- constants
N_NODES = 100000
N_EDGES = 400000
HIDDEN = 128
NODE_FDIM = 133
EDGE_FDIM = 14
DEPTH = 3
NCORES = 8
P = 128
EC = 1024                     # edges per message-phase chunk
EJ = EC // P                  # 8 gathers / transposes per chunk
PK = 8                        # edges packed per msg DRAM row
NPC = N_NODES // NCORES       # 12500 nodes per core
NT = (NPC + P - 1) // P       # 98 node tiles per core
NPAD = NT * P                 # 12544
F32 = mybir.dt.float32
BF = mybir.dt.bfloat16
I32 = mybir.dt.int32
BF_NP = mybir.dt.np(BF)
AF = mybir.ActivationFunctionType
ALU = mybir.AluOpType


# ------------------------------------------------ walrus sync-wait limit
def _split_multi_waits(nc):
    """This container's walrus encodes at most ONE sync-wait per
    instruction (any ISA struct). Tile attaches several. Split: insert a
    NoOp per extra wait immediately before the instruction on the same
    engine (sequencer stalls on each in turn)."""
    n_split = 0
    for f in nc.m.functions:
        for bb in f.blocks:
            out = []
            for ins in bb.instructions:
                si = getattr(ins, "sync_info", None)
                waits = list(si.on_wait) if si is not None else []
                if len(waits) > 1:
                    for k, w in enumerate(waits[:-1]):
                        out.append(mybir.InstNoOp(
                            name=f"{ins.name}.w{k}",
                            sync_info=mybir.SyncInfo(on_wait=[w], on_update=[]),
                            bass_nofuse=True,
                            engine=ins.engine,
                        ))
                        n_split += 1
                    ins.sync_info = mybir.SyncInfo(
                        on_wait=[waits[-1]], on_update=list(si.on_update)
                    )
                out.append(ins)
            bb.instructions = out
    return n_split


# ------------------------------------------------------------- host prep
def _prep(inputs):
    """Shard / reorder inputs on the host. Returns (in_maps, meta)."""
    x = np.ascontiguousarray(np.asarray(inputs["x"], np.float32))
    ea = np.ascontiguousarray(np.asarray(inputs["edge_attr"], np.float32))
    ei = np.asarray(inputs["edge_index"])
    src = np.asarray(ei[0], np.int64)
    tgt = np.asarray(ei[1], np.int64)

    f64 = np.float64
    Wm1 = np.asarray(inputs["Wm1"], f64)
    bm1 = np.asarray(inputs["bm1"], f64)
    Wm2 = np.asarray(inputs["Wm2"], f64)
    bm2 = np.asarray(inputs["bm2"], f64)
    W_ih = np.asarray(inputs["W_ih"], f64)
    b_ih = np.asarray(inputs["b_ih"], f64)
    W_hh = np.asarray(inputs["W_hh"], f64)
    b_hh = np.asarray(inputs["b_hh"], f64)
    Wn = np.asarray(inputs["Wn"], f64)
    bn = np.asarray(inputs["bn"], f64)
    Wo1 = np.asarray(inputs["Wo1"], f64)
    bo1 = np.asarray(inputs["bo1"], f64)
    Wo2 = np.asarray(inputs["Wo2"], f64)
    bo2 = np.asarray(inputs["bo2"], f64)

    H = HIDDEN
    # Fuse Wm2 into the GRU input projection:
    #   gi = h1 @ (Wm2 @ W_ih.T) + (W_ih @ bm2 + b_ih)
    W2G = Wm2 @ W_ih.T                     # [128, 384]
    b2g = W_ih @ bm2 + b_ih                # [384]
    bhh_r, bhh_z, bhh_n = b_hh[:H], b_hh[H:2 * H], b_hh[2 * H:]
    b2g_r, b2g_z, b2g_n = b2g[:H], b2g[H:2 * H], b2g[2 * H:]

    # Wm1 row order for the on-chip activation layout:
    #   xa (gathered)      = x features 0:128       -> Wm1 rows 14:142
    #   xb (edge stream)   = [x feats 128:133 | ea] -> rows 142:147, 0:14
    #   WC                 = messages               -> rows 147:275
    WA = Wm1[14:142]
    WB19 = np.concatenate([Wm1[142:147], Wm1[0:14]], axis=0)  # [19, 128]
    WC = Wm1[147:275]
    WHH = W_hh.T                            # [128, 384]

    WN1 = Wn[0:128]
    WN25 = Wn[128:133]                      # [5, 128]
    WNM = Wn[133:261]

    def bfc(a):
        return np.ascontiguousarray(np.asarray(a, np.float32).astype(BF_NP))

    def col(v):
        return np.ascontiguousarray(
            np.asarray(v, f64).reshape(128, 1).astype(np.float32))

    weights = {
        "WA": bfc(WA), "WB19": bfc(WB19), "WC": bfc(WC),
        "W2G": bfc(W2G), "WHH": bfc(WHH),
        "WN1": bfc(WN1), "WN25": bfc(WN25), "WNM": bfc(WNM),
        "WO1": bfc(Wo1), "WO2": bfc(Wo2),
        "IDN": bfc(np.eye(128)),
        "IOTA": bfc(np.tile(np.arange(128, dtype=f64), (128, 1))),
        "IOTA8": bfc(np.tile(np.arange(128, dtype=f64), (128, PK))),
        "BM1": col(bm1),
        "BR": col(b2g_r + bhh_r),
        "BZP": col(b2g_z + bhh_z),
        "BZN": col(-(b2g_z + bhh_z)),
        "BGN": col(b2g_n),
        "BHN": col(bhh_n),
        "BN": col(bn), "BO1": col(bo1), "BO2": col(bo2),
    }
    x_hi = np.asarray(x[:, 128:133], np.float32)

    # ---- edge sharding by target-node range
    order = np.argsort(tgt, kind="stable")
    tgt_s = tgt[order]
    src_s = src[order]
    bounds = np.searchsorted(tgt_s, NPC * np.arange(NCORES + 1))
    ecounts = np.diff(bounds)
    EPAD = int(np.ceil(ecounts.max() / EC) * EC)
    CH = EPAD // EC
    R8 = EPAD // PK

    in_maps = []
    all_rlo8 = []
    for c in range(NCORES):
        lo, hi = bounds[c], bounds[c + 1]
        ec = hi - lo
        tl = tgt_s[lo:hi] - NPC * c
        rp = np.searchsorted(tl, P * np.arange(NT + 1))

        xaT = np.zeros((128, EPAD), np.float32)
        xaT[:, :ec] = x[src_s[lo:hi], 0:128].T

        eaT = np.zeros((19, EPAD), np.float32)
        eaT[0:5, :ec] = x_hi[src_s[lo:hi]].T
        eaT[5:19, :ec] = ea[order[lo:hi]].T

        # aggregation gather rows + relative segment ids (8-packed rows)
        r_lo8 = rp[:-1] // PK                      # [NT]
        nrows8 = (rp[1:] + PK - 1) // PK - r_lo8
        assert nrows8.max() <= P, f"tile span too large: {nrows8.max()}"
        all_rlo8.append(r_lo8)
        tlp = np.full(EPAD, 1 << 30, np.int64)
        tlp[:ec] = tl
        rows = r_lo8[None, :] + np.arange(P)[:, None]       # [P, NT]
        valid = rows * PK < rp[1:][None, :]
        rows_c = np.where(valid, rows, 0).astype(np.int32)
        e = rows_c[:, :, None] * PK + np.arange(PK)[None, None, :]  # [P,NT,8]
        seg = tlp[np.minimum(e, EPAD - 1)] - P * np.arange(NT)[None, :, None]
        ok = valid[:, :, None] & (seg >= 0) & (seg < P)
        aggseg = np.where(ok, seg, -1).astype(np.float32).reshape(P, NT * PK)
        aggidx = np.ascontiguousarray(rows_c)               # [P, NT] i32

        xT = np.zeros((133, NPAD), np.float32)
        xT[:, :NPC] = x[NPC * c:NPC * (c + 1)].T

        m = {
            "xaT": np.ascontiguousarray(xaT.astype(BF_NP)),
            "eaT": np.ascontiguousarray(eaT.astype(BF_NP)),
            "aggidx": aggidx,
            "aggseg": np.ascontiguousarray(aggseg),
            "xT": np.ascontiguousarray(xT.astype(BF_NP)),
        }
        m.update(weights)
        in_maps.append(m)

    # Per-tile window of message chunks whose DRAM writes the aggregation
    # gather must wait for (Tile does not track DRAM RAW dependencies).
    rlo8 = np.stack(all_rlo8)                      # [NCORES, NT]
    clo = (PK * rlo8.min(axis=0)) // EC
    chi = np.minimum((PK * rlo8.max(axis=0) + PK * P - 1) // EC, CH - 1)
    meta = {"EPAD": EPAD, "CH": CH,
            "aggwin": list(zip(clo.tolist(), chi.tolist()))}
    return in_maps, meta


# ------------------------------------------------------------ bass program
def _build(meta):
    EPAD, CH = meta["EPAD"], meta["CH"]
    R8 = EPAD // PK
    nc = bass.Bass()

    xaT_e = nc.dram_tensor("xaT", [128, EPAD], BF, kind="ExternalInput")
    eaT_e = nc.dram_tensor("eaT", [19, EPAD], BF, kind="ExternalInput")
    aggidx_e = nc.dram_tensor("aggidx", [P, NT], I32, kind="ExternalInput")
    aggseg_e = nc.dram_tensor("aggseg", [P, NT * PK], F32, kind="ExternalInput")
    xT_e = nc.dram_tensor("xT", [133, NPAD], BF, kind="ExternalInput")
    w128 = ["WA", "WC", "WN1", "WNM", "WO1", "WO2", "IDN", "IOTA"]
    w_e = {n: nc.dram_tensor(n, [128, 128], BF, kind="ExternalInput")
           for n in w128}
    w_e["IOTA8"] = nc.dram_tensor("IOTA8", [128, PK * 128], BF,
                                  kind="ExternalInput")
    w_e["WB19"] = nc.dram_tensor("WB19", [19, 128], BF, kind="ExternalInput")
    w_e["WN25"] = nc.dram_tensor("WN25", [5, 128], BF, kind="ExternalInput")
    w_e["W2G"] = nc.dram_tensor("W2G", [128, 384], BF, kind="ExternalInput")
    w_e["WHH"] = nc.dram_tensor("WHH", [128, 384], BF, kind="ExternalInput")
    bnames = ["BM1", "BR", "BZP", "BZN", "BGN", "BHN", "BN", "BO1", "BO2"]
    b_e = {n: nc.dram_tensor(n, [128, 1], F32, kind="ExternalInput")
           for n in bnames}
    out_e = nc.dram_tensor("out", [HIDDEN, NPAD], F32, kind="ExternalOutput")
    msg_e = nc.dram_tensor("msg", [EPAD, HIDDEN], BF,
                           kind="ExternalOutput" if DEBUG_MSG else "Internal")
    nmdbg_e = (nc.dram_tensor("nmdbg", [128, NPAD], F32, kind="ExternalOutput")
               if DEBUG_MSG else None)

    # edge-major message buffer viewed as 8-edge-packed rows for gathers
    msg8 = msg_e[:].rearrange("(r k) h -> r (k h)", k=PK)
    # chunk-c view matching the transposed SBUF layout [p, j, h]
    msg_w = msg_e[:].rearrange("(c j p) h -> c p j h", j=EJ, p=P)

    with tile.TileContext(nc) as tc, ExitStack() as es:
        cst = es.enter_context(tc.tile_pool(name="cst", bufs=1))
        W = {}
        for n, t_e in w_e.items():
            W[n] = cst.tile(list(t_e.shape), BF, tag=n, name=n)
            nc.sync.dma_start(W[n][:], t_e[:])
        B = {}
        for n in bnames:
            B[n] = cst.tile([128, 1], F32, tag=n, name=n)
            nc.sync.dma_start(B[n][:], b_e[n][:])
        aggidx = cst.tile([P, NT], I32, tag="aggidx")
        nc.sync.dma_start(aggidx[:], aggidx_e[:])
        aggseg = cst.tile([P, NT * PK], F32, tag="aggseg")
        nc.sync.dma_start(aggseg[:], aggseg_e[:])

        gp = es.enter_context(tc.tile_pool(name="gp", bufs=2 * EJ))
        ap = es.enter_context(tc.tile_pool(name="ap", bufs=3))
        hp = es.enter_context(tc.tile_pool(name="hp", bufs=3))
        mp = es.enter_context(tc.tile_pool(name="mp", bufs=3))
        np_ = es.enter_context(tc.tile_pool(name="np", bufs=3))
        # PSUM budget (8 banks): tag g = 3 slots x 2 banks (message gates),
        # tag t = 1 slot x 1 bank (bf16 transposes), tag a = 1 slot x 1 bank
        # (node-phase stages packed into one bank)
        pg = es.enter_context(tc.tile_pool(name="pg", bufs=3, space="PSUM"))
        pt = es.enter_context(tc.tile_pool(name="pt", bufs=2, space="PSUM"))

        def psg():
            return pg.tile([128, EC], F32, tag="g", name="g")

        def pst():
            # bf16 PSUM target for PE transposes (1 bank, shared with agg)
            return pt.tile([128, EC], BF, tag="t", name="t")

        def mmh(ps, lhsT, rhs, start, stop):
            # [128, EC] psum filled as two N=512 halves, each its own
            # accumulation group
            for s in range(0, EC, 512):
                nc.tensor.matmul(ps[:, s:s + 512], lhsT, rhs[:, s:s + 512],
                                 start=start, stop=stop)

        IDN = W["IDN"]
        aggwin = meta["aggwin"]
        # tiles become ready once their last covering chunk has been written
        ready = [[] for _ in range(CH)]
        for t in range(NT):
            ready[aggwin[t][1]].append(t)
        mwr = []

        # ---------------------------- aggregation + node phase (per tile)
        def agg_tile(t):
            g = mp.tile([P, PK * P], BF, tag="mg")
            g_ins = nc.gpsimd.indirect_dma_start(
                out=g[:],
                out_offset=None,
                in_=msg8,
                in_offset=IndirectOffsetOnAxis(
                    ap=aggidx[:, t:t + 1], axis=0
                ),
            )
            for cw in range(aggwin[t][0], aggwin[t][1] + 1):
                tile.add_dep_helper(g_ins.ins, mwr[cw].ins,
                                    reason="msg DRAM RAW")
            oh = mp.tile([P, PK, P], BF, tag="oh")
            segb = aggseg[:, PK * t:PK * (t + 1)]
            nc.vector.tensor_tensor(
                out=oh[:],
                in0=segb.to_broadcast([P, PK, P]),
                in1=W["IOTA8"][:].rearrange("p (j h) -> p j h", j=PK),
                op=ALU.is_equal,
            )
            # all four node-phase psum stages share one bank
            ps_a = pt.tile([P, 512], F32, tag="t", name="a")
            ps_nm, ps_nr, ps_o1, ps_o2 = (ps_a[:, 128 * i:128 * (i + 1)]
                                          for i in range(4))
            for j in range(PK):
                nc.tensor.matmul(ps_nm, g[:, P * j:P * (j + 1)], oh[:, j],
                                 start=(j == 0), stop=(j == PK - 1))
            nm = np_.tile([P, P], BF, tag="nm")
            nc.vector.tensor_copy(out=nm[:], in_=ps_nm)
            if DEBUG_MSG:
                nmf = np_.tile([P, P], F32, tag="nmf")
                nc.vector.tensor_copy(out=nmf[:], in_=ps_nm)
                nc.sync.dma_start(nmdbg_e[:, P * t:P * (t + 1)], nmf[:])
            xt1 = np_.tile([P, P], BF, tag="xt1")
            nc.sync.dma_start(xt1[:], xT_e[0:128, P * t:P * (t + 1)])
            xt2 = np_.tile([5, P], BF, tag="xt2")
            nc.sync.dma_start(xt2[:], xT_e[128:133, P * t:P * (t + 1)])
            nc.tensor.matmul(ps_nr, W["WN1"][:], xt1[:], start=True, stop=False)
            nc.tensor.matmul(ps_nr, W["WN25"][:], xt2[:], start=False, stop=False)
            nc.tensor.matmul(ps_nr, W["WNM"][:], nm[:], start=False, stop=True)
            nr = np_.tile([P, P], BF, tag="nr")
            nc.vector.tensor_scalar_add(nr[:], ps_nr, B["BN"][:])
            nc.tensor.matmul(ps_o1, W["WO1"][:], nr[:], start=True, stop=True)
            s = np_.tile([P, P], BF, tag="s")
            nc.scalar.activation(s[:], ps_o1, AF.Relu, bias=B["BO1"][:])
            nc.tensor.matmul(ps_o2, W["WO2"][:], s[:], start=True, stop=True)
            ob = np_.tile([P, P], F32, tag="ob")
            nc.vector.tensor_scalar_add(ob[:], ps_o2, B["BO2"][:])
            nc.sync.dma_start(out_e[:, P * t:P * (t + 1)], ob[:])

        # ------------------------------------------------ message phase
        for c in range(CH):
            xb = ap.tile([19, EC], BF, tag="xb")
            nc.sync.dma_start(xb[:], eaT_e[:, EC * c:EC * (c + 1)])
            xa = ap.tile([128, EC], BF, tag="xa")
            nc.sync.dma_start(xa[:], xaT_e[:, EC * c:EC * (c + 1)])

            ps_m = psg()
            mmh(ps_m, W["WA"][:], xa[:], True, False)
            mmh(ps_m, W["WB19"][:], xb[:], False, True)
            baseS = hp.tile([128, EC], BF, tag="baseS")
            nc.vector.tensor_scalar_add(baseS[:], ps_m, B["BM1"][:])

            h = None
            for d in range(DEPTH):
                if d == 0:
                    ps_dgn = ps_m      # reuse the slot for the gn group
                    h1 = hp.tile([128, EC], BF, tag="h1")
                    nc.vector.tensor_scalar_max(h1[:], baseS[:], 0.0)
                else:
                    ps_dgn = psg()
                    mmh(ps_dgn, W["WC"][:], h[:], True, False)
                    mmh(ps_dgn, IDN[:], baseS[:], False, True)
                    h1 = hp.tile([128, EC], BF, tag="h1")
                    nc.scalar.activation(h1[:], ps_dgn, AF.Relu)

                if d > 0:
                    ps_hn = psg()
                    mmh(ps_hn, W["WHH"][:, 256:384], h[:], True, True)

                ps_gr = psg()
                mmh(ps_gr, W["W2G"][:, 0:128], h1[:], True, d == 0)
                if d > 0:
                    mmh(ps_gr, W["WHH"][:, 0:128], h[:], False, True)
                ps_gz = psg()
                mmh(ps_gz, W["W2G"][:, 128:256], h1[:], True, d == 0)
                if d > 0:
                    mmh(ps_gz, W["WHH"][:, 128:256], h[:], False, True)
                r = hp.tile([128, EC], BF, tag="r")
                nc.scalar.activation(r[:], ps_gr, AF.Sigmoid, bias=B["BR"][:])
                z = hp.tile([128, EC], BF, tag="z")
                if d == 0:
                    # z holds (1 - z_gate) at d0 (h == 0)
                    nc.scalar.activation(
                        z[:], ps_gz, AF.Sigmoid, bias=B["BZN"][:], scale=-1.0
                    )
                else:
                    nc.scalar.activation(
                        z[:], ps_gz, AF.Sigmoid, bias=B["BZP"][:]
                    )
                tt = hp.tile([128, EC], BF, tag="tt")
                if d == 0:
                    nc.vector.tensor_scalar_mul(tt[:], r[:], B["BHN"][:])
                else:
                    nc.vector.scalar_tensor_tensor(
                        tt[:], ps_hn, B["BHN"][:], r[:], ALU.add, ALU.mult
                    )
                mmh(ps_dgn, W["W2G"][:, 256:384], h1[:], True, False)
                mmh(ps_dgn, IDN[:], tt[:], False, True)
                n_t = hp.tile([128, EC], BF, tag="n")
                nc.scalar.activation(n_t[:], ps_dgn, AF.Tanh, bias=B["BGN"][:])
                h_new = hp.tile([128, EC], BF, tag="h")
                if d == 0:
                    nc.vector.tensor_mul(h_new[:], z[:], n_t[:])
                else:
                    c1 = hp.tile([128, EC], BF, tag="c1")
                    nc.vector.tensor_sub(c1[:], h[:], n_t[:])
                    nc.vector.tensor_mul(c1[:], z[:], c1[:])
                    nc.vector.tensor_add(h_new[:], n_t[:], c1[:])
                h = h_new

            psT = pst()
            for j in range(EJ):
                nc.tensor.transpose(
                    psT[:, P * j:P * (j + 1)], h[:, P * j:P * (j + 1)], IDN[:]
                )
            mout = mp.tile([128, EJ, P], BF, tag="mout")
            nc.vector.tensor_copy(
                out=mout[:], in_=psT.rearrange("p (j h) -> p j h", j=EJ)
            )
            mwr.append(nc.sync.dma_start(msg_w[c], mout[:]))
            # emit aggregation for tiles whose message rows are complete
            if c >= 1:
                for t in ready[c - 1]:
                    agg_tile(t)
        for t in ready[CH - 1]:
            agg_tile(t)

    _split_multi_waits(nc)
    return nc


# ---------------------------------------------------------------- kernel
LAST_RESULT = None  # BassKernelResults of the most recent kernel() call
DEBUG_MSG = False   # expose the msg scratch as an output for debugging
USE_BARRIER = False  # all-engine barrier between message and agg phases


def kernel(**inputs) -> np.ndarray:
    global LAST_RESULT
    in_maps, meta = _prep(inputs)
    nc = _build(meta)
    res = run_bass_kernel_spmd(nc, in_maps, list(range(NCORES)))
    LAST_RESULT = res
    out = np.concatenate(
        [np.asarray(res.results[c]["out"])[:, :NPC].T for c in range(NCORES)],
        axis=0,
    )
    return np.ascontiguousarray(out).astype(np.float32)


if __name__ == "__main__":
    sys.path.insert(0, "/root/problem")
    import reference

    inputs = {k: np.asarray(v) for k, v in reference.setup_inputs().items()}
    exp = np.asarray(reference.reference(**inputs))
    act = kernel(**inputs)
    err = np.abs(act - exp).max() / (np.abs(exp).max() + 1e-12)
    print("Relative error:", err)


# revision 39
# speedup vs baseline: 1.1158x; 1.1158x over previous
"""Directed message-passing GNN (chemprop-style D-MPNN) on 8 Trainium2 cores.

Strategy (node-range sharding, zero collectives), v2 — bf16 compute:
  - Host sorts edges by target node and splits nodes into 8 contiguous
    ranges of 12500 (edges follow their target's range, ~E/8 per core).
  - All on-chip compute in bf16 (PSUM accumulation stays fp32): one-pass
    matmuls with fast weight load, 2x DVE elementwise, halved DMA bytes.
  - Per core, per 1024-edge chunk: 8 indirect-DMA gathers of x[:, :128]
    rows (256B bf16 each), PE-transposed to feature-major; the remaining 5
    x features ride a host-prepared [19, E] edge stream together with
    edge_attr.  All DEPTH=3 iterations of the message MLP + GRU run
    on-chip; Wm2 is folded into W_ih on the host.  The (gn + r*ghn) gate
    add runs on the PE via an identity-matmul accumulate into the gn PSUM
    bank instead of a 1x-rate PSUM tensor_tensor.
  - Final messages are PE-transposed to edge-major and written to a DRAM
    scratch buffer (bf16) in target-sorted order, 8-edge-packed rows.
  - Segment-sum: per 128-node tile, ONE indirect gather of 128 8-packed
    rows (2KB each) covering the tile's edge span, one-hot matrices built
    with 4x-rate tensor_scalar is_equal(IOTA, seg), then msg.T @ onehot
    accumulated in PSUM -> feature-major node messages.  Node MLP + final
    PE transpose complete the output tile (fp32).
"""

import sys

sys.path.insert(0, "/opt/trn_rl_repo")

import numpy as np
from contextlib import ExitStack

import concourse.bass as bass
import concourse.mybir as mybir
import concourse.tile as tile
from concourse.bass import IndirectOffsetOnAxis
from concourse.bass_utils import run_bass_kernel_spmd

# ---------------------------------------------------------------- constants
N_NODES = 100000
N_EDGES = 400000
HIDDEN = 128
NODE_FDIM = 133
EDGE_FDIM = 14
DEPTH = 3
NCORES = 8
P = 128
EC = 1024                     # edges per message-phase chunk
EJ = EC // P                  # 8 gathers / transposes per chunk
PK = 8                        # edges packed per msg DRAM row
NPC = N_NODES // NCORES       # 12500 nodes per core
NT = (NPC + P - 1) // P       # 98 node tiles per core
NPAD = NT * P                 # 12544
F32 = mybir.dt.float32
BF = mybir.dt.bfloat16
I32 = mybir.dt.int32
BF_NP = mybir.dt.np(BF)
AF = mybir.ActivationFunctionType
ALU = mybir.AluOpType


# ------------------------------------------------ walrus sync-wait limit
def _split_multi_waits(nc):
    """This container's walrus encodes at most ONE sync-wait per
    instruction (any ISA struct). Tile attaches several. Split: insert a
    NoOp per extra wait immediately before the instruction on the same
    engine (sequencer stalls on each in turn)."""
    n_split = 0
    for f in nc.m.functions:
        for bb in f.blocks:
            out = []
            for ins in bb.instructions:
                si = getattr(ins, "sync_info", None)
                waits = list(si.on_wait) if si is not None else []
                if len(waits) > 1:
                    for k, w in enumerate(waits[:-1]):
                        out.append(mybir.InstNoOp(
                            name=f"{ins.name}.w{k}",
                            sync_info=mybir.SyncInfo(on_wait=[w], on_update=[]),
                            bass_nofuse=True,
                            engine=ins.engine,
                        ))
                        n_split += 1
                    ins.sync_info = mybir.SyncInfo(
                        on_wait=[waits[-1]], on_update=list(si.on_update)
                    )
                out.append(ins)
            bb.instructions = out
    return n_split


# ------------------------------------------------------------- host prep
def _prep(inputs):
    """Shard / reorder inputs on the host. Returns (in_maps, meta)."""
    x = np.ascontiguousarray(np.asarray(inputs["x"], np.float32))
    ea = np.ascontiguousarray(np.asarray(inputs["edge_attr"], np.float32))
    ei = np.asarray(inputs["edge_index"])
    src = np.asarray(ei[0], np.int64)
    tgt = np.asarray(ei[1], np.int64)

    f64 = np.float64
    Wm1 = np.asarray(inputs["Wm1"], f64)
    bm1 = np.asarray(inputs["bm1"], f64)
    Wm2 = np.asarray(inputs["Wm2"], f64)
    bm2 = np.asarray(inputs["bm2"], f64)
    W_ih = np.asarray(inputs["W_ih"], f64)
    b_ih = np.asarray(inputs["b_ih"], f64)
    W_hh = np.asarray(inputs["W_hh"], f64)
    b_hh = np.asarray(inputs["b_hh"], f64)
    Wn = np.asarray(inputs["Wn"], f64)
    bn = np.asarray(inputs["bn"], f64)
    Wo1 = np.asarray(inputs["Wo1"], f64)
    bo1 = np.asarray(inputs["bo1"], f64)
    Wo2 = np.asarray(inputs["Wo2"], f64)
    bo2 = np.asarray(inputs["bo2"], f64)

    H = HIDDEN
    # Fuse Wm2 into the GRU input projection:
    #   gi = h1 @ (Wm2 @ W_ih.T) + (W_ih @ bm2 + b_ih)
    W2G = Wm2 @ W_ih.T                     # [128, 384]
    b2g = W_ih @ bm2 + b_ih                # [384]
    bhh_r, bhh_z, bhh_n = b_hh[:H], b_hh[H:2 * H], b_hh[2 * H:]
    b2g_r, b2g_z, b2g_n = b2g[:H], b2g[H:2 * H], b2g[2 * H:]

    # Wm1 row order for the on-chip activation layout:
    #   xa (gathered)      = x features 0:128       -> Wm1 rows 14:142
    #   xb (edge stream)   = [x feats 128:133 | ea] -> rows 142:147, 0:14
    #   WC                 = messages               -> rows 147:275
    WA = Wm1[14:142]
    WB19 = np.concatenate([Wm1[142:147], Wm1[0:14]], axis=0)  # [19, 128]
    WC = Wm1[147:275]
    WHH = W_hh.T                            # [128, 384]

    WN1 = Wn[0:128]
    WN25 = Wn[128:133]                      # [5, 128]
    WNM = Wn[133:261]

    def bfc(a):
        return np.ascontiguousarray(np.asarray(a, np.float32).astype(BF_NP))

    def col(v):
        return np.ascontiguousarray(
            np.asarray(v, f64).reshape(128, 1).astype(np.float32))

    weights = {
        "WA": bfc(WA), "WB19": bfc(WB19), "WC": bfc(WC),
        "W2G": bfc(W2G), "WHH": bfc(WHH),
        "WN1": bfc(WN1), "WN25": bfc(WN25), "WNM": bfc(WNM),
        "WO1": bfc(Wo1), "WO2": bfc(Wo2),
        "IDN": bfc(np.eye(128)),
        "IOTA": bfc(np.tile(np.arange(128, dtype=f64), (128, 1))),
        "IOTA8": bfc(np.tile(np.arange(128, dtype=f64), (128, PK))),
        "BM1": col(bm1),
        "BR": col(b2g_r + bhh_r),
        "BZP": col(b2g_z + bhh_z),
        "BZN": col(-(b2g_z + bhh_z)),
        "BGN": col(b2g_n),
        "BHN": col(bhh_n),
        "BN": col(bn), "BO1": col(bo1), "BO2": col(bo2),
    }
    x_hi = np.asarray(x[:, 128:133], np.float32)

    # ---- edge sharding by target-node range
    order = np.argsort(tgt, kind="stable")
    tgt_s = tgt[order]
    src_s = src[order]
    bounds = np.searchsorted(tgt_s, NPC * np.arange(NCORES + 1))
    ecounts = np.diff(bounds)
    EPAD = int(np.ceil(ecounts.max() / EC) * EC)
    CH = EPAD // EC
    R8 = EPAD // PK

    in_maps = []
    all_rlo8 = []
    for c in range(NCORES):
        lo, hi = bounds[c], bounds[c + 1]
        ec = hi - lo
        tl = tgt_s[lo:hi] - NPC * c
        rp = np.searchsorted(tl, P * np.arange(NT + 1))

        xaT = np.zeros((128, EPAD), np.float32)
        xaT[:, :ec] = x[src_s[lo:hi], 0:128].T

        eaT = np.zeros((19, EPAD), np.float32)
        eaT[0:5, :ec] = x_hi[src_s[lo:hi]].T
        eaT[5:19, :ec] = ea[order[lo:hi]].T

        # aggregation gather rows + relative segment ids (8-packed rows)
        r_lo8 = rp[:-1] // PK                      # [NT]
        nrows8 = (rp[1:] + PK - 1) // PK - r_lo8
        assert nrows8.max() <= P, f"tile span too large: {nrows8.max()}"
        all_rlo8.append(r_lo8)
        tlp = np.full(EPAD, 1 << 30, np.int64)
        tlp[:ec] = tl
        rows = r_lo8[None, :] + np.arange(P)[:, None]       # [P, NT]
        valid = rows * PK < rp[1:][None, :]
        rows_c = np.where(valid, rows, 0).astype(np.int32)
        e = rows_c[:, :, None] * PK + np.arange(PK)[None, None, :]  # [P,NT,8]
        seg = tlp[np.minimum(e, EPAD - 1)] - P * np.arange(NT)[None, :, None]
        ok = valid[:, :, None] & (seg >= 0) & (seg < P)
        aggseg = np.where(ok, seg, -1).astype(np.float32).reshape(P, NT * PK)
        aggidx = np.ascontiguousarray(rows_c)               # [P, NT] i32

        xT = np.zeros((133, NPAD), np.float32)
        xT[:, :NPC] = x[NPC * c:NPC * (c + 1)].T

        m = {
            "xaT": np.ascontiguousarray(xaT.astype(BF_NP)),
            "eaT": np.ascontiguousarray(eaT.astype(BF_NP)),
            "aggidx": aggidx,
            "aggseg": np.ascontiguousarray(aggseg),
            "xT": np.ascontiguousarray(xT.astype(BF_NP)),
        }
        m.update(weights)
        in_maps.append(m)

    # Per-tile window of message chunks whose DRAM writes the aggregation
    # gather must wait for (Tile does not track DRAM RAW dependencies).
    rlo8 = np.stack(all_rlo8)                      # [NCORES, NT]
    clo = (PK * rlo8.min(axis=0)) // EC
    chi = np.minimum((PK * rlo8.max(axis=0) + PK * P - 1) // EC, CH - 1)
    meta = {"EPAD": EPAD, "CH": CH,
            "aggwin": list(zip(clo.tolist(), chi.tolist()))}
    return in_maps, meta


# ------------------------------------------------------------ bass program
def _build(meta):
    EPAD, CH = meta["EPAD"], meta["CH"]
    R8 = EPAD // PK
    nc = bass.Bass()

    xaT_e = nc.dram_tensor("xaT", [128, EPAD], BF, kind="ExternalInput")
    eaT_e = nc.dram_tensor("eaT", [19, EPAD], BF, kind="ExternalInput")
    aggidx_e = nc.dram_tensor("aggidx", [P, NT], I32, kind="ExternalInput")
    aggseg_e = nc.dram_tensor("aggseg", [P, NT * PK], F32, kind="ExternalInput")
    xT_e = nc.dram_tensor("xT", [133, NPAD], BF, kind="ExternalInput")
    w128 = ["WA", "WC", "WN1", "WNM", "WO1", "WO2", "IDN", "IOTA"]
    w_e = {n: nc.dram_tensor(n, [128, 128], BF, kind="ExternalInput")
           for n in w128}
    w_e["IOTA8"] = nc.dram_tensor("IOTA8", [128, PK * 128], BF,
                                  kind="ExternalInput")
    w_e["WB19"] = nc.dram_tensor("WB19", [19, 128], BF, kind="ExternalInput")
    w_e["WN25"] = nc.dram_tensor("WN25", [5, 128], BF, kind="ExternalInput")
    w_e["W2G"] = nc.dram_tensor("W2G", [128, 384], BF, kind="ExternalInput")
    w_e["WHH"] = nc.dram_tensor("WHH", [128, 384], BF, kind="ExternalInput")
    bnames = ["BM1", "BR", "BZP", "BZN", "BGN", "BHN", "BN", "BO1", "BO2"]
    b_e = {n: nc.dram_tensor(n, [128, 1], F32, kind="ExternalInput")
           for n in bnames}
    out_e = nc.dram_tensor("out", [HIDDEN, NPAD], F32, kind="ExternalOutput")
    msg_e = nc.dram_tensor("msg", [EPAD, HIDDEN], BF,
                           kind="ExternalOutput" if DEBUG_MSG else "Internal")
    nmdbg_e = (nc.dram_tensor("nmdbg", [128, NPAD], F32, kind="ExternalOutput")
               if DEBUG_MSG else None)

    # edge-major message buffer viewed as 8-edge-packed rows for gathers
    msg8 = msg_e[:].rearrange("(r k) h -> r (k h)", k=PK)
    # chunk-c view matching the transposed SBUF layout [p, j, h]
    msg_w = msg_e[:].rearrange("(c j p) h -> c p j h", j=EJ, p=P)

    with tile.TileContext(nc) as tc, ExitStack() as es:
        cst = es.enter_context(tc.tile_pool(name="cst", bufs=1))
        W = {}
        for n, t_e in w_e.items():
            W[n] = cst.tile(list(t_e.shape), BF, tag=n, name=n)
            nc.sync.dma_start(W[n][:], t_e[:])
        B = {}
        for n in bnames:
            B[n] = cst.tile([128, 1], F32, tag=n, name=n)
            nc.sync.dma_start(B[n][:], b_e[n][:])
        aggidx = cst.tile([P, NT], I32, tag="aggidx")
        nc.sync.dma_start(aggidx[:], aggidx_e[:])
        aggseg = cst.tile([P, NT * PK], F32, tag="aggseg")
        nc.sync.dma_start(aggseg[:], aggseg_e[:])

        gp = es.enter_context(tc.tile_pool(name="gp", bufs=2 * EJ))
        ap = es.enter_context(tc.tile_pool(name="ap", bufs=3))
        hp = es.enter_context(tc.tile_pool(name="hp", bufs=3))
        mp = es.enter_context(tc.tile_pool(name="mp", bufs=3))
        np_ = es.enter_context(tc.tile_pool(name="np", bufs=3))
        # PSUM budget (8 banks): tag g = 3 slots x 2 banks (message gates),
        # tag t = 1 slot x 1 bank (bf16 transposes), tag a = 1 slot x 1 bank
        # (node-phase stages packed into one bank)
        pg = es.enter_context(tc.tile_pool(name="pg", bufs=3, space="PSUM"))
        pt = es.enter_context(tc.tile_pool(name="pt", bufs=2, space="PSUM"))

        def psg():
            return pg.tile([128, EC], F32, tag="g", name="g")

        def pst():
            # bf16 PSUM target for PE transposes (1 bank, shared with agg)
            return pt.tile([128, EC], BF, tag="t", name="t")

        def mmh(ps, lhsT, rhs, start, stop):
            # [128, EC] psum filled as two N=512 halves, each its own
            # accumulation group
            for s in range(0, EC, 512):
                nc.tensor.matmul(ps[:, s:s + 512], lhsT, rhs[:, s:s + 512],
                                 start=start, stop=stop)

        IDN = W["IDN"]
        aggwin = meta["aggwin"]
        # tiles become ready once their last covering chunk has been written
        ready = [[] for _ in range(CH)]
        for t in range(NT):
            ready[aggwin[t][1]].append(t)
        mwr = []

        # ---------------------------- aggregation + node phase (per tile)
        def agg_tile(t):
            g = mp.tile([P, PK * P], BF, tag="mg")
            g_ins = nc.gpsimd.indirect_dma_start(
                out=g[:],
                out_offset=None,
                in_=msg8,
                in_offset=IndirectOffsetOnAxis(
                    ap=aggidx[:, t:t + 1], axis=0
                ),
            )
            for cw in range(aggwin[t][0], aggwin[t][1] + 1):
                tile.add_dep_helper(g_ins.ins, mwr[cw].ins,
                                    reason="msg DRAM RAW")
            oh = mp.tile([P, PK, P], BF, tag="oh")
            segb = aggseg[:, PK * t:PK * (t + 1)]
            nc.vector.tensor_tensor(
                out=oh[:],
                in0=segb.to_broadcast([P, PK, P]),
                in1=W["IOTA8"][:].rearrange("p (j h) -> p j h", j=PK),
                op=ALU.is_equal,
            )
            # all four node-phase psum stages share one bank
            ps_a = pt.tile([P, 512], F32, tag="t", name="a")
            ps_nm, ps_nr, ps_o1, ps_o2 = (ps_a[:, 128 * i:128 * (i + 1)]
                                          for i in range(4))
            for j in range(PK):
                nc.tensor.matmul(ps_nm, g[:, P * j:P * (j + 1)], oh[:, j],
                                 start=(j == 0), stop=(j == PK - 1))
            nm = np_.tile([P, P], BF, tag="nm")
            nc.vector.tensor_copy(out=nm[:], in_=ps_nm)
            if DEBUG_MSG:
                nmf = np_.tile([P, P], F32, tag="nmf")
                nc.vector.tensor_copy(out=nmf[:], in_=ps_nm)
                nc.sync.dma_start(nmdbg_e[:, P * t:P * (t + 1)], nmf[:])
            xt1 = np_.tile([P, P], BF, tag="xt1")
            nc.sync.dma_start(xt1[:], xT_e[0:128, P * t:P * (t + 1)])
            xt2 = np_.tile([5, P], BF, tag="xt2")
            nc.sync.dma_start(xt2[:], xT_e[128:133, P * t:P * (t + 1)])
            nc.tensor.matmul(ps_nr, W["WN1"][:], xt1[:], start=True, stop=False)
            nc.tensor.matmul(ps_nr, W["WN25"][:], xt2[:], start=False, stop=False)
            nc.tensor.matmul(ps_nr, W["WNM"][:], nm[:], start=False, stop=True)
            nr = np_.tile([P, P], BF, tag="nr")
            nc.vector.tensor_scalar_add(nr[:], ps_nr, B["BN"][:])
            nc.tensor.matmul(ps_o1, W["WO1"][:], nr[:], start=True, stop=True)
            s = np_.tile([P, P], BF, tag="s")
            nc.scalar.activation(s[:], ps_o1, AF.Relu, bias=B["BO1"][:])
            nc.tensor.matmul(ps_o2, W["WO2"][:], s[:], start=True, stop=True)
            ob = np_.tile([P, P], F32, tag="ob")
            nc.vector.tensor_scalar_add(ob[:], ps_o2, B["BO2"][:])
            nc.sync.dma_start(out_e[:, P * t:P * (t + 1)], ob[:])

        # ------------------------------------------------ message phase
        for c in range(CH):
            xb = ap.tile([19, EC], BF, tag="xb")
            nc.sync.dma_start(xb[:], eaT_e[:, EC * c:EC * (c + 1)])
            xa = ap.tile([128, EC], BF, tag="xa")
            nc.sync.dma_start(xa[:], xaT_e[:, EC * c:EC * (c + 1)])

            ps_m = psg()
            mmh(ps_m, W["WA"][:], xa[:], True, False)
            mmh(ps_m, W["WB19"][:], xb[:], False, True)
            baseS = hp.tile([128, EC], BF, tag="baseS")
            nc.vector.tensor_scalar_add(baseS[:], ps_m, B["BM1"][:])

            h = None
            for d in range(DEPTH):
                if d == 0:
                    ps_dgn = ps_m      # reuse the slot for the gn group
                    h1 = hp.tile([128, EC], BF, tag="h1")
                    nc.vector.tensor_scalar_max(h1[:], baseS[:], 0.0)
                else:
                    ps_dgn = psg()
                    mmh(ps_dgn, W["WC"][:], h[:], True, False)
                    mmh(ps_dgn, IDN[:], baseS[:], False, True)
                    h1 = hp.tile([128, EC], BF, tag="h1")
                    nc.scalar.activation(h1[:], ps_dgn, AF.Relu)

                if d > 0:
                    ps_hn = psg()
                    mmh(ps_hn, W["WHH"][:, 256:384], h[:], True, True)
                    hnb = hp.tile([128, EC], BF, tag="hnb")
                    nc.vector.tensor_scalar_add(hnb[:], ps_hn, B["BHN"][:])

                ps_gr = psg()
                mmh(ps_gr, W["W2G"][:, 0:128], h1[:], True, d == 0)
                if d > 0:
                    mmh(ps_gr, W["WHH"][:, 0:128], h[:], False, True)
                ps_gz = psg()
                mmh(ps_gz, W["W2G"][:, 128:256], h1[:], True, d == 0)
                if d > 0:
                    mmh(ps_gz, W["WHH"][:, 128:256], h[:], False, True)
                r = hp.tile([128, EC], BF, tag="r")
                nc.scalar.activation(r[:], ps_gr, AF.Sigmoid, bias=B["BR"][:])
                z = hp.tile([128, EC], BF, tag="z")
                if d == 0:
                    # z holds (1 - z_gate) at d0 (h == 0)
                    nc.scalar.activation(
                        z[:], ps_gz, AF.Sigmoid, bias=B["BZN"][:], scale=-1.0
                    )
                else:
                    nc.scalar.activation(
                        z[:], ps_gz, AF.Sigmoid, bias=B["BZP"][:]
                    )
                tt = hp.tile([128, EC], BF, tag="tt")
                if d == 0:
                    nc.vector.tensor_scalar_mul(tt[:], r[:], B["BHN"][:])
                else:
                    nc.vector.tensor_mul(tt[:], r[:], hnb[:])
                mmh(ps_dgn, W["W2G"][:, 256:384], h1[:], True, False)
                mmh(ps_dgn, IDN[:], tt[:], False, True)
                n_t = hp.tile([128, EC], BF, tag="n")
                nc.scalar.activation(n_t[:], ps_dgn, AF.Tanh, bias=B["BGN"][:])
                h_new = hp.tile([128, EC], BF, tag="h")
                if d == 0:
                    nc.vector.tensor_mul(h_new[:], z[:], n_t[:])
                else:
                    c1 = hp.tile([128, EC], BF, tag="c1")
                    nc.vector.tensor_sub(c1[:], h[:], n_t[:])
                    nc.vector.tensor_mul(c1[:], z[:], c1[:])
                    nc.vector.tensor_add(h_new[:], n_t[:], c1[:])
                h = h_new

            psT = pst()
            for j in range(EJ):
                nc.tensor.transpose(
                    psT[:, P * j:P * (j + 1)], h[:, P * j:P * (j + 1)], IDN[:]
                )
            mout = mp.tile([128, EJ, P], BF, tag="mout")
            nc.vector.tensor_copy(
                out=mout[:], in_=psT.rearrange("p (j h) -> p j h", j=EJ)
            )
            mwr.append(nc.sync.dma_start(msg_w[c], mout[:]))
            # emit aggregation for tiles whose message rows are complete
            if c >= 1:
                for t in ready[c - 1]:
                    agg_tile(t)
        for t in ready[CH - 1]:
            agg_tile(t)

    _split_multi_waits(nc)
    return nc


# ---------------------------------------------------------------- kernel
LAST_RESULT = None  # BassKernelResults of the most recent kernel() call
DEBUG_MSG = False   # expose the msg scratch as an output for debugging
USE_BARRIER = False  # all-engine barrier between message and agg phases


def kernel(**inputs) -> np.ndarray:
    global LAST_RESULT
    in_maps, meta = _prep(inputs)
    nc = _build(meta)
    res = run_bass_kernel_spmd(nc, in_maps, list(range(NCORES)))
    LAST_RESULT = res
    out = np.concatenate(
        [np.asarray(res.results[c]["out"])[:, :NPC].T for c in range(NCORES)],
        axis=0,
    )
    return np.ascontiguousarray(out).astype(np.float32)


if __name__ == "__main__":
    sys.path.insert(0, "/root/problem")
    import reference

    inputs = {k: np.asarray(v) for k, v in reference.setup_inputs().items()}
    exp = np.asarray(reference.reference(**inputs))
    act = kernel(**inputs)
    err = np.abs(act - exp).max() / (np.abs(exp).max() + 1e-12)
    print("Relative error:", err)
